# revision 1
# baseline (speedup 1.0000x reference)
"""Trainium2 Bass kernel for MetaBayesLinearParallel.

Math (per sample s):
    W[s]  = weight_mu + weight_sigma * eps_w[s]          # (OUT, IN)
    Bv[s] = bias_mu + bias_sigma * eps_b[s]              # (OUT,)
    out[s] = x[s] @ W[s].T + Bv[s]                       # (B, OUT)

Sharding over 8 cores: 2-way split of the samples axis x 4-way split of
OUT.  Each core handles S_PC=4 samples and O_PC=512 output rows, which
minimizes per-core HBM traffic (16MB eps + 8MB x + 8MB mu/sigma = 32MB).

Per-core pipeline (all compute in bf16, fp32 PSUM accumulation):
  once:  sigma tiles cast-loaded;  muT = transpose(mu);  xT[s] = transpose(x[s])
  per sample:
    se   = sigma * eps_w[s]                       (DVE, bf16 2x mode)
    WT_i = transpose(se)_i + muT_i                (PE transpose + DVE add)
    psum[b,:] = sum_i xT_i[:,b].T @ WT_i  (+ ones.T @ Bv via K=1 matmul)
    out[s,b,:] = psum                             (ACT copy + DMA store)
"""

from contextlib import ExitStack

import numpy as np

import concourse.bacc as bacc
import concourse.mybir as mybir
import concourse.tile as tile
from concourse.bass_utils import run_bass_kernel_spmd
from concourse.masks import make_identity

P = 128
S, B, IN, OUT = 8, 256, 2048, 2048
SAMPLE_WAYS, OUT_WAYS = 2, 4
N_CORES = SAMPLE_WAYS * OUT_WAYS
S_PC = S // SAMPLE_WAYS
O_PC = OUT // OUT_WAYS

BF16 = mybir.dt.bfloat16
F32 = mybir.dt.float32


def build_core_program(s_pc=S_PC, o_pc=O_PC, in_dim=IN, b_dim=B, repeat=1,
                       loop_repeat=0, skip_input_dma=False, pipeline_ib=True):
    """One NeuronCore's program; identical on all cores (SPMD over slices)."""
    o_tiles = o_pc // P
    i_blks = in_dim // P
    b_tiles = b_dim // P

    nc = bacc.Bacc("TRN2")
    x_d = nc.declare_dram_parameter("x", [s_pc, b_dim, in_dim], F32, isOutput=False)
    eps_d = nc.declare_dram_parameter("eps_w", [s_pc, o_pc, in_dim], F32, isOutput=False)
    mu_d = nc.declare_dram_parameter("mu", [o_pc, in_dim], F32, isOutput=False)
    sig_d = nc.declare_dram_parameter("sigma", [o_pc, in_dim], F32, isOutput=False)
    bmu_d = nc.declare_dram_parameter("bias_mu", [1, o_pc], F32, isOutput=False)
    bsig_d = nc.declare_dram_parameter("bias_sigma", [1, o_pc], F32, isOutput=False)
    epsb_d = nc.declare_dram_parameter("eps_b", [s_pc, o_pc], F32, isOutput=False)
    out_d = nc.declare_dram_parameter("out", [s_pc, b_dim, o_pc], F32, isOutput=True)

    with ExitStack() as ctx:
        tc = ctx.enter_context(tile.TileContext(nc))
        consts = ctx.enter_context(tc.tile_pool(name="consts", bufs=1))
        resident = ctx.enter_context(tc.tile_pool(name="resident", bufs=1))
        ld = ctx.enter_context(tc.tile_pool(name="ld", bufs=4))
        eps_pool = ctx.enter_context(tc.tile_pool(name="eps_pool", bufs=4))
        xb_pool = ctx.enter_context(tc.tile_pool(name="xb_pool", bufs=2))
        wt_pool = ctx.enter_context(tc.tile_pool(name="wt", bufs=4))
        outp = ctx.enter_context(tc.tile_pool(name="outp", bufs=4))
        ps_tr = ctx.enter_context(tc.tile_pool(name="ps_tr", bufs=3, space="PSUM"))
        ps_xt = ctx.enter_context(tc.tile_pool(name="ps_xt", bufs=2, space="PSUM"))
        ps_out = ctx.enter_context(tc.tile_pool(name="ps_out", bufs=3, space="PSUM"))

        ident = consts.tile([P, P], BF16)
        make_identity(nc, ident)
        ident32 = consts.tile([P, P], F32)
        make_identity(nc, ident32)
        ones = consts.tile([1, P], BF16)
        nc.vector.memset(ones[:], 1.0)

        args = (nc, tc, consts, resident, ld, eps_pool, xb_pool, wt_pool, outp,
                ps_tr, ps_xt, ps_out, ident, ident32, ones,
                x_d, eps_d, mu_d, sig_d, bmu_d, bsig_d, epsb_d, out_d,
                s_pc, o_pc, in_dim, b_dim, o_tiles, i_blks, b_tiles)
        if loop_repeat:
            with tc.For_i(0, loop_repeat, 1):
                _kernel_body(*args, 0, skip_input_dma, pipeline_ib)
        else:
            for _rep in range(repeat):
                _kernel_body(*args, _rep, skip_input_dma, pipeline_ib)

    nc.compile()
    return nc


def _kernel_body(nc, tc, consts, resident, ld, eps_pool, xb_pool, wt_pool, outp,
                 ps_tr, ps_xt, ps_out, ident, ident32, ones,
                 x_d, eps_d, mu_d, sig_d, bmu_d, bsig_d, epsb_d, out_d,
                 s_pc, o_pc, in_dim, b_dim, o_tiles, i_blks, b_tiles, rep,
                 skip_input_dma=False, pipeline_ib=True):
    BF16 = mybir.dt.bfloat16
    F32 = mybir.dt.float32

    def in_dma(out, in_):
        if not skip_input_dma:
            nc.gpsimd.dma_start(out=out, in_=in_)
        else:
            nc.gpsimd.memset(out, 0.25)

    # ---------------- input DMA issue order (SWDGE queue is FIFO) ---------
    # x[s0] -> mu -> sigma -> eps[s0] -> (x[s], eps[s]) round robin
    xb_tiles = []
    eps_tiles = {}

    def load_x(s):
        xb = xb_pool.tile([P, b_tiles, in_dim], BF16, tag="xb", name=f"xb_{rep}_{s}")
        in_dma(xb[:], x_d[s, :, :].rearrange("(a p) i -> p a i", p=P))
        xb_tiles.append(xb)

    i_spans = min(4, i_blks)
    span = in_dim // i_spans

    def load_eps(s):
        # i-major spans: the compute for i-block ib only needs the span
        # containing ib, so the tail sample's wt/matmul pipeline overlaps
        # its own eps arrival.
        ep = eps_pool.tile([P, o_tiles, in_dim], BF16, tag="eps_ld", name=f"eps_{rep}_{s}")
        for isp in range(i_spans):
            in_dma(ep[:, :, isp * span:(isp + 1) * span],
                   eps_d[s, :, isp * span:(isp + 1) * span]
                   .rearrange("(a p) i -> p a i", p=P))
        eps_tiles[s] = ep

    load_x(0)
    mu_all = resident.tile([P, o_tiles, in_dim], BF16, tag="mu_ld", name=f"mu_{rep}")
    in_dma(mu_all[:], mu_d[:, :].rearrange("(a p) i -> p a i", p=P))
    sigma_sb = resident.tile([P, o_tiles, in_dim], BF16, tag="sigma", name=f"sigma_{rep}")
    in_dma(sigma_sb[:], sig_d[:, :].rearrange("(a p) i -> p a i", p=P))
    load_eps(0)
    for s in range(1, s_pc):
        load_x(s)
        load_eps(s)

    # bias inputs (tiny, HWDGE)
    bmu_sb = consts.tile([1, o_pc], F32, tag="bmu", name=f"bmu_{rep}")
    nc.sync.dma_start(out=bmu_sb[:], in_=bmu_d[:, :])
    bsig_sb = consts.tile([1, o_pc], F32, tag="bsig", name=f"bsig_{rep}")
    nc.sync.dma_start(out=bsig_sb[:], in_=bsig_d[:, :])
    epsb_sb = consts.tile([1, s_pc * o_pc], F32, tag="epsb", name=f"epsb_{rep}")
    nc.sync.dma_start(out=epsb_sb[:], in_=epsb_d[:, :])

    # ---------------- transposed-layout builders --------------------------
    xT_all = resident.tile([P, s_pc, i_blks, b_dim], BF16, tag="xT", name=f"xT_{rep}")

    def build_xT(s):
        for ib in range(i_blks):
            pxt = ps_xt.tile([P, b_dim], BF16, tag="ps_xt")
            for bt in range(b_tiles):
                nc.tensor.transpose(
                    pxt[:, bt * P:(bt + 1) * P],
                    xb_tiles[s][:, bt, ib * P:(ib + 1) * P], ident[:])
            nc.scalar.copy(xT_all[:, s, ib, :], pxt[:])

    # xT[0] first (its x arrives first), then muT (needed by every sample's
    # wt add); xT for later samples is interleaved into the compute loop so
    # the static PE program order never waits on late x arrivals.
    build_xT(0)

    muT_sb = resident.tile([P, i_blks, o_pc], BF16, tag="muT", name=f"muT_{rep}")
    for ib in range(i_blks):
        pmu = ps_tr.tile([P, o_pc], BF16, tag="ps_seT")
        for ot in range(o_tiles):
            nc.tensor.transpose(
                pmu[:, ot * P:(ot + 1) * P],
                mu_all[:, ot, ib * P:(ib + 1) * P], ident[:])
        nc.scalar.copy(muT_sb[:, ib, :], pmu[:])
    build_xT(1)

    # ---------------- per-sample compute ---------------------------------
    def make_bias(s):
        btmp = ld.tile([1, o_pc], F32, tag="btmp")
        nc.vector.tensor_mul(btmp[:], bsig_sb[:], epsb_sb[:, s * o_pc:(s + 1) * o_pc])
        bv = ld.tile([1, o_pc], BF16, tag="bv", name=f"bv_{rep}_{s}")
        nc.vector.tensor_add(bv[:], bmu_sb[:], btmp[:])
        bv_tiles[s] = bv

    bv_tiles = {}

    def se_mul(s, isp):
        sl = slice(isp * span, (isp + 1) * span)
        nc.vector.tensor_mul(eps_tiles[s][:, :, sl], eps_tiles[s][:, :, sl],
                             sigma_sb[:, :, sl])

    for isp in range(i_spans):
        se_mul(0, isp)
    make_bias(0)

    for s in range(s_pc):
        se = eps_tiles[s]
        psum_out = []
        for bt in range(b_tiles):
            po = ps_out.tile([P, o_pc], F32, tag="ps_out", name=f"ps_out_{rep}_{s}_{bt}")
            psum_out.append(po)

        def seT_group(ib):
            pseT = ps_tr.tile([P, o_pc], BF16, tag="ps_seT", name=f"pseT_{rep}_{s}_{ib}")
            for ot in range(o_tiles):
                nc.tensor.transpose(
                    pseT[:, ot * P:(ot + 1) * P], se[:, ot, ib * P:(ib + 1) * P], ident[:])
            return pseT

        # software-pipelined: PE emits the NEXT i-block's transposes before
        # this i-block's matmuls, so the DVE wt-add latency is hidden.
        pseT_cur = seT_group(0) if pipeline_ib else None
        for ib in range(i_blks):
            if not pipeline_ib:
                pseT_cur = seT_group(ib)
            wt = wt_pool.tile([P, o_pc], BF16, tag="wt")
            nc.vector.tensor_add(wt[:], pseT_cur[:], muT_sb[:, ib, :])
            # interleave next sample's se muls into this sample's DVE stream,
            # timed for when its eps spans have arrived
            _q = i_blks // i_spans
            if s + 1 < s_pc and ib % _q == (1 if _q > 1 else 0):
                isp2 = ib // _q
                if isp2 < i_spans:
                    se_mul(s + 1, isp2)
                    if isp2 == i_spans - 1:
                        make_bias(s + 1)
            if pipeline_ib and ib + 1 < i_blks:
                pseT_cur = seT_group(ib + 1)
            for bt in range(b_tiles):
                nc.tensor.matmul(
                    psum_out[bt][:], xT_all[:, s, ib, bt * P:(bt + 1) * P], wt[:],
                    start=(ib == 0), stop=False)
        for bt in range(b_tiles):
            nc.tensor.matmul(psum_out[bt][:], ones[:], bv_tiles[s][:], start=False, stop=True)
            o_sb = outp.tile([P, o_pc], F32, tag="o_sb")
            nc.scalar.copy(o_sb[:], psum_out[bt][:])
            nc.sync.dma_start(out=out_d[s, bt * P:(bt + 1) * P, :], in_=o_sb[:])
        if s + 2 < s_pc:
            build_xT(s + 2)


_prog_cache = {}
_last_in_maps = None


def _get_program(key):
    if key not in _prog_cache:
        _prog_cache[key] = build_core_program(*key)
    return _prog_cache[key]


def kernel(x, weight_mu, weight_sigma, bias_mu, bias_sigma, eps_w, eps_b):
    global _last_in_maps
    x = np.ascontiguousarray(x, dtype=np.float32)
    weight_mu = np.ascontiguousarray(weight_mu, dtype=np.float32)
    weight_sigma = np.ascontiguousarray(weight_sigma, dtype=np.float32)
    bias_mu = np.ascontiguousarray(bias_mu, dtype=np.float32)
    bias_sigma = np.ascontiguousarray(bias_sigma, dtype=np.float32)
    eps_w = np.ascontiguousarray(eps_w, dtype=np.float32)
    eps_b = np.ascontiguousarray(eps_b, dtype=np.float32)

    nc = _get_program((S_PC, O_PC, IN, B))

    in_maps = []
    for c in range(N_CORES):
        sg, og = divmod(c, OUT_WAYS)
        s_lo, s_hi = sg * S_PC, (sg + 1) * S_PC
        o_lo, o_hi = og * O_PC, (og + 1) * O_PC
        in_maps.append({
            "x": x[s_lo:s_hi],
            "eps_w": np.ascontiguousarray(eps_w[s_lo:s_hi, o_lo:o_hi, :]),
            "mu": np.ascontiguousarray(weight_mu[o_lo:o_hi]),
            "sigma": np.ascontiguousarray(weight_sigma[o_lo:o_hi]),
            "bias_mu": bias_mu[o_lo:o_hi].reshape(1, O_PC),
            "bias_sigma": bias_sigma[o_lo:o_hi].reshape(1, O_PC),
            "eps_b": np.ascontiguousarray(eps_b[s_lo:s_hi, o_lo:o_hi]),
        })

    _last_in_maps = in_maps
    res = run_bass_kernel_spmd(nc, in_maps, core_ids=list(range(N_CORES)))

    out = np.empty((S, B, OUT), dtype=np.float32)
    for c in range(N_CORES):
        sg, og = divmod(c, OUT_WAYS)
        out[sg * S_PC:(sg + 1) * S_PC, :, og * O_PC:(og + 1) * O_PC] = res.results[c]["out"]
    return out



# revision 7
# speedup vs baseline: 1.1183x; 1.1183x over previous
"""Trainium2 Bass kernel for MetaBayesLinearParallel.

Math (per sample s):
    W[s]  = weight_mu + weight_sigma * eps_w[s]          # (OUT, IN)
    Bv[s] = bias_mu + bias_sigma * eps_b[s]              # (OUT,)
    out[s] = x[s] @ W[s].T + Bv[s]                       # (B, OUT)

Sharding over 8 cores: 2-way split of the samples axis x 4-way split of
OUT.  Each core handles S_PC=4 samples and O_PC=512 output rows, which
minimizes per-core HBM traffic (16MB eps + 8MB x + 8MB mu/sigma = 32MB).

Per-core pipeline (all compute in bf16, fp32 PSUM accumulation):
  inputs cast-loaded to bf16 by SWDGE DMA (measured at line rate).
  W built in NATURAL layout on DVE (both ops SBUF+SBUF, 2x mode):
      w = eps; w *= sigma; w += mu        (in-place in the eps tile)
  then per i-block: PE transpose w -> PSUM, ACT copy -> SBUF wt,
  PE matmul psum[b,:] += xT_i[:,b].T @ wt (+ ones.T @ Bv K=1 matmul).
"""

from contextlib import ExitStack

import numpy as np

import concourse.bacc as bacc
import concourse.mybir as mybir
import concourse.tile as tile
from concourse.bass_utils import run_bass_kernel_spmd
from concourse.masks import make_identity

P = 128
S, B, IN, OUT = 8, 256, 2048, 2048
SAMPLE_WAYS, OUT_WAYS = 2, 4
N_CORES = SAMPLE_WAYS * OUT_WAYS
S_PC = S // SAMPLE_WAYS
O_PC = OUT // OUT_WAYS

BF16 = mybir.dt.bfloat16
F32 = mybir.dt.float32


def build_core_program(s_pc=S_PC, o_pc=O_PC, in_dim=IN, b_dim=B, repeat=1,
                       skip_input_dma=False, tr_ahead=2):
    """One NeuronCore's program; identical on all cores (SPMD over slices)."""
    o_tiles = o_pc // P
    i_blks = in_dim // P
    b_tiles = b_dim // P
    i_spans = 4
    span = in_dim // i_spans          # 512
    ibs_per_span = i_blks // i_spans  # 4

    nc = bacc.Bacc("TRN2")
    x_d = nc.declare_dram_parameter("x", [s_pc, b_dim, in_dim], F32, isOutput=False)
    eps_d = nc.declare_dram_parameter("eps_w", [s_pc, o_pc, in_dim], F32, isOutput=False)
    mu_d = nc.declare_dram_parameter("mu", [o_pc, in_dim], F32, isOutput=False)
    sig_d = nc.declare_dram_parameter("sigma", [o_pc, in_dim], F32, isOutput=False)
    bmu_d = nc.declare_dram_parameter("bias_mu", [1, o_pc], F32, isOutput=False)
    bsig_d = nc.declare_dram_parameter("bias_sigma", [1, o_pc], F32, isOutput=False)
    epsb_d = nc.declare_dram_parameter("eps_b", [s_pc, o_pc], F32, isOutput=False)
    out_d = nc.declare_dram_parameter("out", [s_pc, b_dim, o_pc], F32, isOutput=True)

    with ExitStack() as ctx:
        tc = ctx.enter_context(tile.TileContext(nc))
        consts = ctx.enter_context(tc.tile_pool(name="consts", bufs=1))
        resident = ctx.enter_context(tc.tile_pool(name="resident", bufs=1))
        ld = ctx.enter_context(tc.tile_pool(name="ld", bufs=4))
        eps_pool = ctx.enter_context(tc.tile_pool(name="eps_pool", bufs=4))
        xb_pool = ctx.enter_context(tc.tile_pool(name="xb_pool", bufs=2))
        wt_pool = ctx.enter_context(tc.tile_pool(name="wt", bufs=2 * tr_ahead + 2))
        outp = ctx.enter_context(tc.tile_pool(name="outp", bufs=4))
        ps_tr = ctx.enter_context(tc.tile_pool(name="ps_tr", bufs=tr_ahead + 2,
                                               space="PSUM"))
        ps_xt = ctx.enter_context(tc.tile_pool(name="ps_xt", bufs=1, space="PSUM"))
        ps_out = ctx.enter_context(tc.tile_pool(name="ps_out", bufs=3, space="PSUM"))

        ident = consts.tile([P, P], BF16)
        make_identity(nc, ident)
        ones = consts.tile([1, P], BF16)
        nc.vector.memset(ones[:], 1.0)

        for rep in range(repeat):
            _kernel_body(nc, tc, consts, resident, ld, eps_pool, xb_pool,
                         wt_pool, outp, ps_tr, ps_xt, ps_out, ident, ones,
                         x_d, eps_d, mu_d, sig_d, bmu_d, bsig_d, epsb_d, out_d,
                         s_pc, o_pc, in_dim, b_dim, o_tiles, i_blks, b_tiles,
                         i_spans, span, ibs_per_span, rep,
                         skip_input_dma, tr_ahead)

    nc.compile()
    return nc


def _kernel_body(nc, tc, consts, resident, ld, eps_pool, xb_pool, wt_pool,
                 outp, ps_tr, ps_xt, ps_out, ident, ones,
                 x_d, eps_d, mu_d, sig_d, bmu_d, bsig_d, epsb_d, out_d,
                 s_pc, o_pc, in_dim, b_dim, o_tiles, i_blks, b_tiles,
                 i_spans, span, ibs_per_span, rep, skip_input_dma, tr_ahead):
    def in_dma(out, in_):
        if not skip_input_dma:
            nc.gpsimd.dma_start(out=out, in_=in_)
        else:
            nc.gpsimd.memset(out, 0.25)

    # ---------------- input DMA issue order (SWDGE queue is FIFO) ---------
    # x0 | per span: sigma, mu, eps0 | x1, eps1 | x2, eps2 | x3, eps3
    # so sample 0's span-pipeline starts after ~5MB instead of ~14MB.
    xb_tiles = []
    eps_tiles = {}

    def load_x(s):
        xb = xb_pool.tile([P, b_tiles, in_dim], BF16, tag="xb", name=f"xb_{rep}_{s}")
        in_dma(xb[:], x_d[s, :, :].rearrange("(a p) i -> p a i", p=P))
        xb_tiles.append(xb)

    def load_eps_span(s, isp):
        sl = slice(isp * span, (isp + 1) * span)
        in_dma(eps_tiles[s][:, :, sl],
               eps_d[s, :, sl].rearrange("(a p) i -> p a i", p=P))

    sigma_sb = resident.tile([P, o_tiles, in_dim], BF16, tag="sigma", name=f"sigma_{rep}")
    mu_sb = resident.tile([P, o_tiles, in_dim], BF16, tag="mu", name=f"mu_{rep}")

    load_x(0)
    eps_tiles[0] = eps_pool.tile([P, o_tiles, in_dim], BF16, tag="eps_ld",
                                 name=f"eps_{rep}_0")
    for isp in range(i_spans):
        sl = slice(isp * span, (isp + 1) * span)
        in_dma(sigma_sb[:, :, sl], sig_d[:, sl].rearrange("(a p) i -> p a i", p=P))
        in_dma(mu_sb[:, :, sl], mu_d[:, sl].rearrange("(a p) i -> p a i", p=P))
        load_eps_span(0, isp)
        if isp == 0 and s_pc > 1:
            load_x(1)  # early: sample 1's xT builds mid-sample-0
    for s in range(1, s_pc):
        if s > 1:
            load_x(s)
        eps_tiles[s] = eps_pool.tile([P, o_tiles, in_dim], BF16, tag="eps_ld",
                                     name=f"eps_{rep}_{s}")
        for isp in range(i_spans):
            load_eps_span(s, isp)

    # bias inputs (tiny, HWDGE)
    bmu_sb = consts.tile([1, o_pc], F32, tag="bmu", name=f"bmu_{rep}")
    nc.sync.dma_start(out=bmu_sb[:], in_=bmu_d[:, :])
    bsig_sb = consts.tile([1, o_pc], F32, tag="bsig", name=f"bsig_{rep}")
    nc.sync.dma_start(out=bsig_sb[:], in_=bsig_d[:, :])
    epsb_sb = consts.tile([1, s_pc * o_pc], F32, tag="epsb", name=f"epsb_{rep}")
    nc.sync.dma_start(out=epsb_sb[:], in_=epsb_d[:, :])

    # ---------------- xT builder -----------------------------------------
    xT_all = resident.tile([P, s_pc, i_blks, b_dim], BF16, tag="xT", name=f"xT_{rep}")

    def build_xT(s, ib_lo, ib_hi):
        for ib in range(ib_lo, ib_hi):
            pxt = ps_xt.tile([P, b_dim], BF16, tag="ps_xt")
            for bt in range(b_tiles):
                nc.tensor.transpose(
                    pxt[:, bt * P:(bt + 1) * P],
                    xb_tiles[s][:, bt, ib * P:(ib + 1) * P], ident[:])
            nc.vector.tensor_copy(xT_all[:, s, ib, :], pxt[:])

    # ---------------- per-sample compute ---------------------------------
    bv_tiles = {}

    def make_bias(s):
        btmp = ld.tile([1, o_pc], F32, tag="btmp")
        nc.vector.tensor_mul(btmp[:], bsig_sb[:], epsb_sb[:, s * o_pc:(s + 1) * o_pc])
        bv = ld.tile([1, o_pc], BF16, tag="bv", name=f"bv_{rep}_{s}")
        nc.vector.tensor_add(bv[:], bmu_sb[:], btmp[:])
        bv_tiles[s] = bv

    built_spans = set()
    bias_done = set()

    def ensure_w_span(s, isp):
        # in-place: eps tile becomes W = mu + sigma*eps (natural layout)
        if (s, isp) in built_spans:
            return
        built_spans.add((s, isp))
        sl = slice(isp * span, (isp + 1) * span)
        w = eps_tiles[s]
        nc.vector.tensor_mul(w[:, :, sl], w[:, :, sl], sigma_sb[:, :, sl])
        nc.vector.tensor_add(w[:, :, sl], w[:, :, sl], mu_sb[:, :, sl])

    def ensure_bias(s):
        if s not in bias_done:
            bias_done.add(s)
            make_bias(s)

    def tr_group(s, ib):
        # W^T for one i-block: PSUM [128(i), o_pc]
        ensure_w_span(s, ib // ibs_per_span)
        w = eps_tiles[s]
        pwT = ps_tr.tile([P, o_pc], BF16, tag="ps_wT", name=f"pwT_{rep}_{s}_{ib}")
        for ot in range(o_tiles):
            nc.tensor.transpose(
                pwT[:, ot * P:(ot + 1) * P], w[:, ot, ib * P:(ib + 1) * P], ident[:])
        wt = wt_pool.tile([P, o_pc], BF16, tag="wt")
        nc.scalar.copy(wt[:], pwT[:])
        return wt

    # prologue: xT for sample 0; W spans are built just-in-time.
    build_xT(0, 0, i_blks)
    ensure_bias(0)

    for s in range(s_pc):
        psum_out = []
        for bt in range(b_tiles):
            po = ps_out.tile([P, o_pc], F32, tag="ps_out", name=f"ps_out_{rep}_{s}_{bt}")
            psum_out.append(po)

        # software pipeline: keep tr_ahead i-blocks of wT in flight.
        wt_q = [tr_group(s, ib) for ib in range(min(tr_ahead, i_blks))] if s == 0 else wt_q

        for ib in range(i_blks):
            # stage future DVE / PE-transpose work so queues stay fed
            nxt = ib + tr_ahead
            if nxt < i_blks:
                # DVE runs one span ahead of the PE transposes
                isp_pre = nxt // ibs_per_span + 1
                if nxt % ibs_per_span == 0 and isp_pre < i_spans:
                    ensure_w_span(s, isp_pre)
                # build next sample's xT in the second half of this sample
                # (x[s+1]'s DMA lands mid-sample; earlier would stall PE)
                if s + 1 < s_pc and ib in (i_blks // 2 + 1, i_blks // 2 + 5):
                    q = 0 if ib == i_blks // 2 + 1 else i_blks // 2
                    build_xT(s + 1, q, q + i_blks // 2)
                wt_q.append(tr_group(s, nxt))
            elif s + 1 < s_pc:
                k = nxt - i_blks
                if k == 0:
                    ensure_bias(s + 1)
                if k < tr_ahead:
                    wt_q.append(tr_group(s + 1, k))
            wt = wt_q.pop(0)
            for bt in range(b_tiles):
                nc.tensor.matmul(
                    psum_out[bt][:], xT_all[:, s, ib, bt * P:(bt + 1) * P], wt[:],
                    start=(ib == 0), stop=False)
        for bt in range(b_tiles):
            nc.tensor.matmul(psum_out[bt][:], ones[:], bv_tiles[s][:],
                             start=False, stop=True)
            o_sb = outp.tile([P, o_pc], F32, tag="o_sb")
            nc.scalar.copy(o_sb[:], psum_out[bt][:])
            nc.sync.dma_start(out=out_d[s, bt * P:(bt + 1) * P, :], in_=o_sb[:])


_prog_cache = {}
_last_in_maps = None


def _get_program(key):
    if key not in _prog_cache:
        _prog_cache[key] = build_core_program(*key)
    return _prog_cache[key]


def kernel(x, weight_mu, weight_sigma, bias_mu, bias_sigma, eps_w, eps_b):
    global _last_in_maps
    x = np.ascontiguousarray(x, dtype=np.float32)
    weight_mu = np.ascontiguousarray(weight_mu, dtype=np.float32)
    weight_sigma = np.ascontiguousarray(weight_sigma, dtype=np.float32)
    bias_mu = np.ascontiguousarray(bias_mu, dtype=np.float32)
    bias_sigma = np.ascontiguousarray(bias_sigma, dtype=np.float32)
    eps_w = np.ascontiguousarray(eps_w, dtype=np.float32)
    eps_b = np.ascontiguousarray(eps_b, dtype=np.float32)

    nc = _get_program((S_PC, O_PC, IN, B))

    in_maps = []
    for c in range(N_CORES):
        sg, og = divmod(c, OUT_WAYS)
        s_lo, s_hi = sg * S_PC, (sg + 1) * S_PC
        o_lo, o_hi = og * O_PC, (og + 1) * O_PC
        in_maps.append({
            "x": x[s_lo:s_hi],
            "eps_w": np.ascontiguousarray(eps_w[s_lo:s_hi, o_lo:o_hi, :]),
            "mu": np.ascontiguousarray(weight_mu[o_lo:o_hi]),
            "sigma": np.ascontiguousarray(weight_sigma[o_lo:o_hi]),
            "bias_mu": bias_mu[o_lo:o_hi].reshape(1, O_PC),
            "bias_sigma": bias_sigma[o_lo:o_hi].reshape(1, O_PC),
            "eps_b": np.ascontiguousarray(eps_b[s_lo:s_hi, o_lo:o_hi]),
        })

    _last_in_maps = in_maps
    res = run_bass_kernel_spmd(nc, in_maps, core_ids=list(range(N_CORES)))

    out = np.empty((S, B, OUT), dtype=np.float32)
    for c in range(N_CORES):
        sg, og = divmod(c, OUT_WAYS)
        out[sg * S_PC:(sg + 1) * S_PC, :, og * O_PC:(og + 1) * O_PC] = res.results[c]["out"]
    return out
